# revision 1
# baseline (speedup 1.0000x reference)
"""Multi-head attention (B=4, N=2048, C=768, H=12) on 8 trn2 NeuronCores.

Sharding: core c handles batch b = c//2 and query rows [ (c%2)*1024, +1024 ).
Each core computes K/V for its full batch (duplicated across the pair),
attention for all 12 heads over its 1024 queries, and the output projection
for its rows. Output gather is pure concatenation (no cross-core reduce).

On-chip layout (per core):
  xt  = x_b.T           [768, 2048]   (c on partitions)
  QT  = Wq @ xq.T       [768, 1024]   head h rows h*64..h*64+63
  KT  = Wk @ x.T        [768, 2048]
  V   = x @ Wv.T        [2048, 780]   row-tiles of 128 keys; head h in cols
                                      h*65..h*65+63, col h*65+64 == 1.0 (ones
                                      column -> PV matmul also emits softmax
                                      denominators)
  ST_h = (K_h Q_h^T)    [128k, 1024q] PSUM per k-tile; exp on ScalarE with
                                      the 1/8 attention scale folded in
  OT_h = [V_h|1]^T P_h  [65, 1024]    PSUM accumulated over 16 k-tiles;
                                      row 64 = softmax denominators
  Y    = OT^T Wp^T + bp [1024, 768]
"""

import os
import sys

import numpy as np

sys.path.insert(0, "/opt/trn_rl_repo")

import concourse.bass as bass
from concourse import bacc
import concourse.mybir as mybir
from concourse.tile import TileContext
from concourse.bass_utils import run_bass_kernel_spmd
from concourse.dma_utils import dma_copy

P = 128
C = 768
NK = 2048
NQ = 1024
H = 12
DH = 64
CT = C // P          # 6 c-tiles (contraction tiles for the linears)
KT = NK // P         # 16 key tiles
QCH = 512            # q-chunk (max psum bank free dim for fp32)
NQC = NQ // QCH      # 2 q chunks
SCALE = DH ** -0.5
F32 = mybir.dt.float32
BF16 = mybir.dt.bfloat16

LAST_RESULT = None
_PROG = None


def _build_program() -> bass.Bass:
    nc = bacc.Bacc(None, target_bir_lowering=False)

    xt = nc.dram_tensor("xt", [C, NK], F32, kind="ExternalInput")
    xqt = nc.dram_tensor("xqt", [C, NQ], F32, kind="ExternalInput")
    wqt = nc.dram_tensor("wqt", [C, C], F32, kind="ExternalInput")
    wkt = nc.dram_tensor("wkt", [C, C], F32, kind="ExternalInput")
    wvt = nc.dram_tensor("wvt", [C, C], F32, kind="ExternalInput")
    wpt = nc.dram_tensor("wpt", [C, C], F32, kind="ExternalInput")
    bp = nc.dram_tensor("bp", [1, C], F32, kind="ExternalInput")
    y = nc.dram_tensor("y", [NQ, C], F32, kind="ExternalOutput")

    with TileContext(nc) as tc:
        with (
            tc.tile_pool(name="persist", bufs=1) as persist,
            tc.tile_pool(name="pt", bufs=6) as ptp,
            tc.tile_pool(name="small", bufs=2) as small,
            tc.tile_pool(name="ysb", bufs=2) as ysb,
            tc.tile_pool(name="psa", bufs=2, space="PSUM") as psa,
            tc.tile_pool(name="psb", bufs=2, space="PSUM") as psb,
        ):
            # ---- load weights/activations, casting to bf16 in the DMA ----
            def load_cast(dram, rows, cols, tag):
                tiles = []
                for i in range(rows // P):
                    t = persist.tile([P, cols], BF16, tag=f"{tag}{i}")
                    dma_copy(nc.gpsimd, t[:, :], dram[i * P:(i + 1) * P, :])
                    tiles.append(t)
                return tiles

            xtb = load_cast(xt, C, NK, "xtb")
            xqb = load_cast(xqt, C, NQ, "xqb")
            wqb = load_cast(wqt, C, C, "wqb")
            wkb = load_cast(wkt, C, C, "wkb")
            wvb = load_cast(wvt, C, C, "wvb")
            wpb = load_cast(wpt, C, C, "wpb")

            bpb = persist.tile([1, C], BF16, tag="bpb")
            dma_copy(nc.gpsimd, bpb[:, :], bp[:, :])

            ones = persist.tile([P, P], BF16, tag="ones")
            nc.gpsimd.memset(ones[:, :], 1.0)
            onesf = persist.tile([DH, DH], F32, tag="onesf")
            nc.gpsimd.memset(onesf[:, :], 1.0)

            # ---- QT / KT: W @ x.T   [C, n] ----
            def wx(wtiles, xtiles, n, tag):
                out_tiles = []
                for i in range(CT):
                    t = persist.tile([P, n], BF16, tag=f"{tag}{i}")
                    out_tiles.append(t)
                for i in range(CT):          # output row tile (cout)
                    for j in range(n // QCH):  # n chunk
                        ps = psa.tile([P, QCH], F32, tag="a")
                        for k in range(CT):  # contraction c tile
                            nc.tensor.matmul(
                                ps[:, :],
                                lhsT=wtiles[k][:, i * P:(i + 1) * P],
                                rhs=xtiles[k][:, j * QCH:(j + 1) * QCH],
                                start=(k == 0),
                                stop=(k == CT - 1),
                            )
                        nc.vector.tensor_copy(
                            out_tiles[i][:, j * QCH:(j + 1) * QCH], ps[:, :]
                        )
                return out_tiles

            qtb = wx(wqb, xqb, NQ, "qt")
            ktb = wx(wkb, xtb, NK, "kt")

            # ---- V = x @ Wv.T  [2048, 12*65], ones column per head ----
            vtb = []
            for i in range(KT):
                t = persist.tile([P, H * (DH + 1)], BF16, tag=f"v{i}")
                nc.gpsimd.memset(t[:, :], 1.0)
                vtb.append(t)
            for i in range(KT):              # key row tile
                for (c0, csz) in ((0, QCH), (QCH, C - QCH)):
                    ps = psb.tile([P, csz], F32, tag="b")
                    for k in range(CT):
                        nc.tensor.matmul(
                            ps[:, :],
                            lhsT=xtb[k][:, i * P:(i + 1) * P],
                            rhs=wvb[k][:, c0:c0 + csz],
                            start=(k == 0),
                            stop=(k == CT - 1),
                        )
                    # scatter heads into stride-65 columns (leaves ones col)
                    nh = csz // DH
                    h0 = c0 // DH
                    dst = vtb[i][:, :].rearrange(
                        "p (h e) -> p h e", e=DH + 1
                    )[:, h0:h0 + nh, 0:DH]
                    src = ps[:, :].rearrange("p (h e) -> p h e", e=DH)
                    nc.vector.tensor_copy(dst, src)

            # ---- attention, head pairs ----
            otb = []   # [hd, q] bf16, tile hp holds heads 2hp (0:64), 2hp+1
            for hp in range(CT):
                t = persist.tile([P, NQ], BF16, tag=f"ot{hp}")
                otb.append(t)

            def norm_dve_front(hp, osb, den):
                # one reciprocal for both heads' denominators, which live at
                # partitions 0 and 32 (engine APs need 32-aligned bases)
                rec = small.tile([DH, NQ], F32, tag="rec")
                nc.vector.reciprocal(rec[0:1, :], den[0:1, :])
                nc.vector.reciprocal(rec[32:33, :], den[32:33, :])
                recb = small.tile([DH, NQ], BF16, tag="recb")
                nc.vector.tensor_copy(recb[0:1, :], rec[0:1, :])
                nc.vector.tensor_copy(recb[32:33, :], rec[32:33, :])
                return recb

            def norm_tail(hp, osb, recs):
                # PE broadcast of 1/denom + fused normalize-multiply
                for idx, r in ((0, recs[0]), (1, recs[1])):
                    ob = 32 * idx
                    rb = psa.tile([DH, NQ], F32, tag="a")
                    for j in range(NQC):
                        nc.tensor.matmul(
                            rb[:, j * QCH:(j + 1) * QCH],
                            lhsT=ones[ob:ob + 1, 0:DH],
                            rhs=r[0:1, j * QCH:(j + 1) * QCH],
                            start=True, stop=True,
                        )
                    nc.vector.tensor_mul(
                        otb[hp][idx * DH:(idx + 1) * DH, :],
                        osb[idx * DH:(idx + 1) * DH, :],
                        rb[0:DH, :],
                    )

            prev = None   # (hp, osb) awaiting normalization
            for hp in range(CT):
                h0, h1 = 2 * hp, 2 * hp + 1
                if prev is not None:
                    rec = norm_dve_front(*prev)
                    prev_recs = (rec[0:1, :], rec[32:33, :])
                ot0 = psb.tile([DH + 1, NQ], F32, tag="b")
                ot1 = psb.tile([DH + 1, NQ], F32, tag="b")
                def av_pair(i, pt0, pt1):
                    for j in range(NQC):
                        nc.tensor.matmul(
                            ot0[:, j * QCH:(j + 1) * QCH],
                            lhsT=vtb[i][:, h0 * (DH + 1):h0 * (DH + 1) + DH + 1],
                            rhs=pt0[:, j * QCH:(j + 1) * QCH],
                            start=(i == 0), stop=(i == KT - 1),
                        )
                        nc.tensor.matmul(
                            ot1[:, j * QCH:(j + 1) * QCH],
                            lhsT=vtb[i][:, h1 * (DH + 1):h1 * (DH + 1) + DH + 1],
                            rhs=pt1[:, j * QCH:(j + 1) * QCH],
                            start=(i == 0), stop=(i == KT - 1),
                        )

                # software pipeline: AV(i-1) is emitted between ST(i) and
                # exp(i), so the PE never stalls on the exp it just fed
                pending = None
                for i in range(KT):
                    st0 = psa.tile([P, NQ], F32, tag="a")
                    st1 = psa.tile([P, NQ], F32, tag="a")
                    for j in range(NQC):
                        # heads alternate PE row groups (base 0 / base 64)
                        # -> hardware runs the pair concurrently
                        nc.tensor.matmul(
                            st0[:, j * QCH:(j + 1) * QCH],
                            lhsT=ktb[hp][0:DH, i * P:(i + 1) * P],
                            rhs=qtb[hp][0:DH, j * QCH:(j + 1) * QCH],
                            start=True, stop=True,
                            tile_position=(0, 0),
                        )
                        nc.tensor.matmul(
                            st1[:, j * QCH:(j + 1) * QCH],
                            lhsT=ktb[hp][DH:P, i * P:(i + 1) * P],
                            rhs=qtb[hp][DH:P, j * QCH:(j + 1) * QCH],
                            start=True, stop=True,
                            tile_position=(64, 0),
                        )
                    if pending is not None:
                        av_pair(*pending)
                    pt0 = ptp.tile([P, NQ], BF16, tag="pt")
                    pt1 = ptp.tile([P, NQ], BF16, tag="pt")
                    nc.scalar.activation(
                        pt0[:, :], st0[:, :],
                        mybir.ActivationFunctionType.Exp, scale=SCALE,
                    )
                    nc.scalar.activation(
                        pt1[:, :], st1[:, :],
                        mybir.ActivationFunctionType.Exp, scale=SCALE,
                    )
                    pending = (i, pt0, pt1)
                av_pair(*pending)
                if prev is not None:
                    norm_tail(prev[0], prev[1], prev_recs)
                # drain this pair's PSUM fast so the next pair can start
                osb = small.tile([P, NQ], F32, tag="osb")
                den = small.tile([DH, NQ], F32, tag="den")
                nc.vector.tensor_copy(osb[0:DH, :], ot0[0:DH, :])
                nc.vector.tensor_copy(den[0:1, :], ot0[DH:DH + 1, :])
                nc.vector.tensor_copy(osb[DH:P, :], ot1[0:DH, :])
                nc.vector.tensor_copy(den[32:33, :], ot1[DH:DH + 1, :])
                prev = (hp, osb, den)

            # flush the last pair
            rec = norm_dve_front(*prev)
            norm_tail(prev[0], prev[1], (rec[0:1, :], rec[32:33, :]))

            # ---- projection: Y[q, co] = OT.T @ WpT + bp ----
            for qi in range(NQ // P):
                yt = ysb.tile([P, C], F32, tag="y")
                for (c0, csz) in ((0, QCH), (QCH, C - QCH)):
                    ps = psa.tile([P, csz], F32, tag="a")
                    for k in range(CT):
                        nc.tensor.matmul(
                            ps[:, :],
                            lhsT=otb[k][:, qi * P:(qi + 1) * P],
                            rhs=wpb[k][:, c0:c0 + csz],
                            start=(k == 0), stop=False,
                        )
                    nc.tensor.matmul(
                        ps[:, :],
                        lhsT=ones[0:1, 0:P],
                        rhs=bpb[0:1, c0:c0 + csz],
                        start=False, stop=True,
                    )
                    nc.vector.tensor_copy(yt[:, c0:c0 + csz], ps[:, :])
                nc.sync.dma_start(out=y[qi * P:(qi + 1) * P, :], in_=yt[:, :])

    nc.compile()
    return nc


def _get_prog() -> bass.Bass:
    global _PROG
    if _PROG is None:
        _PROG = _build_program()
    return _PROG


def kernel(x, Wq, Wk, Wv, Wp, bp):
    global LAST_RESULT
    x = np.asarray(x, dtype=np.float32)
    wqt = np.ascontiguousarray(np.asarray(Wq, np.float32).T)
    wkt = np.ascontiguousarray(np.asarray(Wk, np.float32).T)
    wvt = np.ascontiguousarray(np.asarray(Wv, np.float32).T)
    wpt = np.ascontiguousarray(np.asarray(Wp, np.float32).T)
    bpv = np.ascontiguousarray(np.asarray(bp, np.float32).reshape(1, C))

    B, N, _ = x.shape
    in_maps = []
    for core in range(8):
        b, qh = core // 2, core % 2
        xt = np.ascontiguousarray(x[b].T)
        xqt = np.ascontiguousarray(xt[:, qh * NQ:(qh + 1) * NQ])
        in_maps.append({
            "xt": xt, "xqt": xqt,
            "wqt": wqt, "wkt": wkt, "wvt": wvt, "wpt": wpt, "bp": bpv,
        })

    res = run_bass_kernel_spmd(
        _get_prog(), in_maps, core_ids=list(range(8)),
        trace=bool(os.environ.get("BASS_TRACE")),
    )
    LAST_RESULT = res

    out = np.empty((B, N, C), np.float32)
    for core in range(8):
        b, qh = core // 2, core % 2
        out[b, qh * NQ:(qh + 1) * NQ, :] = res.results[core]["y"]
    return out



# revision 9
# speedup vs baseline: 1.2837x; 1.2837x over previous
"""Multi-head attention (B=4, N=2048, C=768, H=12) on 8 trn2 NeuronCores.

Sharding: core c handles batch b = c//2 and query rows [ (c%2)*1024, +1024 ).

Per-core engine plan (v2):
  PE     : QKV/proj linears, QK^T (2 heads packed on row groups), PV with the
           two q-chunks packed on column groups (full 128-wide array), ones
           matmuls accumulating softmax denominators on psum rows 0/32/64/96,
           reciprocal-broadcast matmuls.
  ScalarE: exp for even heads (table exp), denominator reciprocal via
           ln -> exp(-x), psum->sbuf drains of the linears.
  VectorE: exp for odd heads via Schraudolph int16 bit-trick (one
           tensor_scalar: i16 = st*A + B, bitcast bf16), attention drains,
           normalize muls.
  DMA    : host-side bf16 inputs; sbuf->sbuf shuffles assemble the normalized
           attention output into projection layout during attention.
  Emission interleaves V (pair 0) and next pair's Q/K (pairs 1-4) into the
  attention loop so the PE never idles and HAM stays at full clock.

PSUM budget (8 banks): st 2x[128,1024] = 4, t 2x[128,512] = 2, den 1, rb 1.
"""

import os
import sys

import numpy as np
import ml_dtypes

sys.path.insert(0, "/opt/trn_rl_repo")

import concourse.bass as bass
from concourse import bacc
import concourse.mybir as mybir
from concourse.tile import TileContext
from concourse.bass_utils import run_bass_kernel_spmd
from concourse.dma_utils import dma_copy

P = 128
C = 768
NK = 2048
NQ = 1024
H = 12
DH = 64
CT = C // P          # 6 c-tiles (contraction tiles for the linears)
KT = NK // P         # 16 key tiles
QCH = 512            # q-chunk (max psum bank free dim for fp32)
SCALE = DH ** -0.5
F32 = mybir.dt.float32
BF16 = mybir.dt.bfloat16
I16 = mybir.dt.int16
EXP = mybir.ActivationFunctionType.Exp
LN = mybir.ActivationFunctionType.Ln

LOG2E = 1.4426950408889634
A16 = 128.0 * LOG2E * SCALE          # fold the 1/8 attention scale
B16 = 127.0 * 128.0 - 0.043677448 * 128.0

LAST_RESULT = None
_PROG = None


def _build_program() -> bass.Bass:
    nc = bacc.Bacc(None, target_bir_lowering=False)

    # host supplies bf16 (halves DMA bytes; matches baseline numerics)
    wqt = nc.dram_tensor("wqt", [C, C], BF16, kind="ExternalInput")
    wkt = nc.dram_tensor("wkt", [C, C], BF16, kind="ExternalInput")
    xq = nc.dram_tensor("xq", [C, NQ], BF16, kind="ExternalInput")
    xt = nc.dram_tensor("xt", [C, NK], BF16, kind="ExternalInput")
    wvt = nc.dram_tensor("wvt", [C, C], BF16, kind="ExternalInput")
    wpt = nc.dram_tensor("wpt", [C, C], BF16, kind="ExternalInput")
    bp = nc.dram_tensor("bp", [1, C], BF16, kind="ExternalInput")
    y = nc.dram_tensor("y", [NQ, C], F32, kind="ExternalOutput")

    with TileContext(nc) as tc:
        with (
            tc.tile_pool(name="persist", bufs=1) as persist,
            tc.tile_pool(name="pt0p", bufs=2) as pt0p,
            tc.tile_pool(name="pt1p", bufs=2) as pt1p,
            tc.tile_pool(name="small", bufs=2) as small,
            tc.tile_pool(name="ysb", bufs=2) as ysb,
            tc.tile_pool(name="ps_st", bufs=2, space="PSUM") as ps_st,
            tc.tile_pool(name="ps_t", bufs=2, space="PSUM") as ps_t,
            tc.tile_pool(name="ps_den", bufs=1, space="PSUM") as ps_den,
            tc.tile_pool(name="ps_rb", bufs=1, space="PSUM") as ps_rb,
        ):
            # ---- load inputs (DMA order = dependency order) ----
            def load(cols, tag):
                return [
                    persist.tile([P, cols], BF16, tag=f"{tag}{i}", name=f"{tag}{i}")
                    for i in range(CT)
                ]

            def dma_tiles(tiles, dram, cols, col0=0):
                for i, t in enumerate(tiles):
                    dma_copy(
                        nc.gpsimd,
                        t[:, col0:col0 + cols],
                        dram[i * P:(i + 1) * P, col0:col0 + cols],
                    )

            wqb = load(C, "wqb")
            wkb = load(C, "wkb")
            xqb = load(NQ, "xqb")
            xtb = load(NK, "xtb")
            wvb = load(C, "wvb")
            wpb = load(C, "wpb")

            dma_tiles(wqb, wqt, C)
            dma_tiles(xqb, xq, NQ)
            dma_tiles(wkb, wkt, C)
            dma_tiles(xtb, xt, NQ, col0=0)      # first half of keys
            dma_tiles(xtb, xt, NQ, col0=NQ)     # second half
            dma_tiles(wvb, wvt, C)
            dma_tiles(wpb, wpt, C)

            bpb = persist.tile([1, C], BF16, tag="bpb")
            dma_copy(nc.gpsimd, bpb[:, :], bp[:, :])

            ones = persist.tile([P, P], BF16, tag="ones")
            nc.gpsimd.memset(ones[:, :], 1.0)

            qtb = [persist.tile([P, NQ], BF16, tag=f"qt{i}", name=f"qt{i}") for i in range(CT)]
            ktb = [persist.tile([P, NK], BF16, tag=f"kt{i}", name=f"kt{i}") for i in range(CT)]
            vtb = [persist.tile([P, C], BF16, tag=f"v{i}", name=f"v{i}") for i in range(KT)]
            otb = [persist.tile([P, NQ], BF16, tag=f"ot{i}", name=f"ot{i}") for i in range(CT)]

            def emit_q(hp, pool):
                # qtb[hp] = Wq rows [hp*128, +128) @ xq  -> [128, 1024]
                for j in range(2):
                    ps = pool.tile([P, QCH], F32, tag="st")
                    for k in range(CT):
                        nc.tensor.matmul(
                            ps[:, :],
                            lhsT=wqb[k][:, hp * P:(hp + 1) * P],
                            rhs=xqb[k][:, j * QCH:(j + 1) * QCH],
                            start=(k == 0), stop=(k == CT - 1),
                        )
                    nc.scalar.copy(qtb[hp][:, j * QCH:(j + 1) * QCH], ps[:, :])

            def emit_k_chunk(hp, j, pool):
                ps = pool.tile([P, QCH], F32, tag="st")
                for k in range(CT):
                    nc.tensor.matmul(
                        ps[:, :],
                        lhsT=wkb[k][:, hp * P:(hp + 1) * P],
                        rhs=xtb[k][:, j * QCH:(j + 1) * QCH],
                        start=(k == 0), stop=(k == CT - 1),
                    )
                nc.scalar.copy(ktb[hp][:, j * QCH:(j + 1) * QCH], ps[:, :])

            def emit_v(i, pool):
                # vtb[i] = x rows [i*128,+128) @ Wv.T -> [128, 768]
                for (c0, csz) in ((0, QCH), (QCH, C - QCH)):
                    ps = pool.tile([P, QCH], F32, tag="st")
                    for k in range(CT):
                        nc.tensor.matmul(
                            ps[:, 0:csz],
                            lhsT=xtb[k][:, i * P:(i + 1) * P],
                            rhs=wvb[k][:, c0:c0 + csz],
                            start=(k == 0), stop=(k == CT - 1),
                        )
                    nc.scalar.copy(vtb[i][:, c0:c0 + csz], ps[:, 0:csz])

            # ---- initial linears: Q/K for pair 0, V tile 0 ----
            emit_q(0, ps_st)
            for j in range(4):
                emit_k_chunk(0, j, ps_st)
            emit_v(0, ps_st)

            def make_fillers(hp):
                fills = []
                if hp == 0:
                    for i in range(1, KT):
                        fills.append(lambda i=i: emit_v(i, ps_rb))
                    fills.append(lambda: emit_q(1, ps_rb))
                    for j in range(4):
                        fills.append(lambda j=j: emit_k_chunk(1, j, ps_rb))
                elif hp < CT - 1:
                    nxt = hp + 1
                    fills.append(lambda: emit_q(nxt, ps_rb))
                    for j in range(4):
                        fills.append(lambda j=j: emit_k_chunk(nxt, j, ps_rb))
                return fills

            for hp in range(CT):
                h0, h1 = 2 * hp, 2 * hp + 1
                fills = make_fillers(hp)
                nf = len(fills)

                den = ps_den.tile([97, QCH], F32, tag="den")
                t_h0 = ps_t.tile([P, QCH], F32, tag="t")
                t_h1 = ps_t.tile([P, QCH], F32, tag="t")

                def av_den(i, pt0, pt1_i):
                    first, last = (i == 0), (i == KT - 1)
                    v0 = vtb[i][:, h0 * DH:(h0 + 1) * DH]
                    v1 = vtb[i][:, h1 * DH:(h1 + 1) * DH]

                    def pch(h, c):
                        sl = slice(c * QCH, (c + 1) * QCH)
                        if h == 0:
                            return pt0[:, sl]
                        return pt1_i[:, sl].bitcast(BF16)

                    # col-group packed: q-chunk 0 -> psum[0:64], chunk 1 -> [64:128]
                    nc.tensor.matmul(
                        t_h0[0:DH, :], lhsT=v0, rhs=pch(0, 0),
                        start=first, stop=last, tile_position=(0, 0),
                    )
                    nc.tensor.matmul(
                        t_h0[DH:P, :], lhsT=v0, rhs=pch(0, 1),
                        start=first, stop=last, tile_position=(0, 64),
                    )
                    nc.tensor.matmul(
                        t_h1[0:DH, :], lhsT=v1, rhs=pch(1, 0),
                        start=first, stop=last, tile_position=(0, 0),
                    )
                    nc.tensor.matmul(
                        t_h1[DH:P, :], lhsT=v1, rhs=pch(1, 1),
                        start=first, stop=last, tile_position=(0, 64),
                    )
                    # denominator rows: h0c0 -> 0, h0c1 -> 32, h1c0 -> 64, h1c1 -> 96
                    for pos, (h, c) in zip(
                        (0, 32, 64, 96), ((0, 0), (0, 1), (1, 0), (1, 1))
                    ):
                        nc.tensor.matmul(
                            den[pos:pos + 1, :],
                            lhsT=ones[:, pos:pos + 1],
                            rhs=pch(h, c),
                            start=first, stop=last, tile_position=(0, pos),
                        )

                pending = None   # (i, pt0, pt1_i) awaiting AV + den
                for i in range(KT):
                    # filler PE work while ST waits on previous exp
                    for f in range(i * nf // KT, (i + 1) * nf // KT):
                        fills[f]()
                    st1 = ps_st.tile([P, NQ], F32, tag="st")
                    st0 = ps_st.tile([P, NQ], F32, tag="st")
                    for j in range(2):
                        nc.tensor.matmul(
                            st1[:, j * QCH:(j + 1) * QCH],
                            lhsT=ktb[hp][DH:P, i * P:(i + 1) * P],
                            rhs=qtb[hp][DH:P, j * QCH:(j + 1) * QCH],
                            start=True, stop=True,
                            tile_position=(64, 0),
                        )
                        nc.tensor.matmul(
                            st0[:, j * QCH:(j + 1) * QCH],
                            lhsT=ktb[hp][0:DH, i * P:(i + 1) * P],
                            rhs=qtb[hp][0:DH, j * QCH:(j + 1) * QCH],
                            start=True, stop=True,
                            tile_position=(0, 0),
                        )
                    if pending is not None:
                        av_den(*pending)
                    # exp: odd head on VectorE (bit trick), even head on ScalarE
                    pt1_i = pt1p.tile([P, NQ], I16, tag="pt1")
                    nc.vector.tensor_scalar(
                        pt1_i[:, :], st1[:, :], A16, B16,
                        mybir.AluOpType.mult, mybir.AluOpType.add,
                    )
                    pt0 = pt0p.tile([P, NQ], BF16, tag="pt0")
                    nc.scalar.activation(pt0[:, :], st0[:, :], EXP, scale=SCALE)
                    pending = (i, pt0, pt1_i)
                av_den(*pending)

                # ---- pair tail: reciprocal, broadcast, normalize, assemble ----
                lnden = small.tile([97, QCH], F32, tag="lnden")
                nc.scalar.activation(lnden[:, :], den[:, :], LN)
                recb = small.tile([97, QCH], BF16, tag="recb")
                nc.scalar.activation(recb[:, :], lnden[:, :], EXP, scale=-1.0)

                for h, t_ps in ((0, t_h0), (1, t_h1)):
                    rb = ps_rb.tile([P, QCH], F32, tag="st")
                    r0, r1 = (0, 32) if h == 0 else (64, 96)
                    nc.tensor.matmul(
                        rb[0:DH, :], lhsT=ones[r0:r0 + 1, 0:DH],
                        rhs=recb[r0:r0 + 1, :],
                        start=True, stop=True, tile_position=(r0, 0),
                    )
                    nc.tensor.matmul(
                        rb[DH:P, :], lhsT=ones[r1:r1 + 1, 0:DH],
                        rhs=recb[r1:r1 + 1, :],
                        start=True, stop=True, tile_position=(r1, 64),
                    )
                    tmp = small.tile([P, QCH], F32, tag="tmp")
                    nc.vector.tensor_copy(tmp[:, :], t_ps[:, :])
                    osb = small.tile([P, QCH], BF16, tag="osb")
                    nc.vector.tensor_mul(osb[:, :], tmp[:, :], rb[:, :])
                    # assemble projection layout: head rows at partition h*64,
                    # q chunks side by side in the free dim
                    nc.sync.dma_start(
                        out=otb[hp][h * DH:(h + 1) * DH, 0:QCH],
                        in_=osb[0:DH, :],
                    )
                    nc.sync.dma_start(
                        out=otb[hp][h * DH:(h + 1) * DH, QCH:NQ],
                        in_=osb[DH:P, :],
                    )

            # ---- projection: Y[q, co] = OT.T @ WpT + bp ----
            for qi in range(NQ // P):
                yt = ysb.tile([P, C], F32, tag="y")
                for (c0, csz) in ((0, QCH), (QCH, C - QCH)):
                    ps = ps_st.tile([P, QCH], F32, tag="st")
                    for k in range(CT):
                        nc.tensor.matmul(
                            ps[:, 0:csz],
                            lhsT=otb[k][:, qi * P:(qi + 1) * P],
                            rhs=wpb[k][:, c0:c0 + csz],
                            start=(k == 0), stop=False,
                        )
                    nc.tensor.matmul(
                        ps[:, 0:csz],
                        lhsT=ones[0:1, 0:P],
                        rhs=bpb[0:1, c0:c0 + csz],
                        start=False, stop=True, tile_position=(0, 0),
                    )
                    nc.scalar.copy(yt[:, c0:c0 + csz], ps[:, 0:csz])
                nc.sync.dma_start(out=y[qi * P:(qi + 1) * P, :], in_=yt[:, :])

    nc.compile()
    return nc


def _get_prog() -> bass.Bass:
    global _PROG
    if _PROG is None:
        _PROG = _build_program()
    return _PROG


def kernel(x, Wq, Wk, Wv, Wp, bp):
    global LAST_RESULT
    bf = ml_dtypes.bfloat16
    x = np.asarray(x, dtype=np.float32)
    wqt = np.ascontiguousarray(np.asarray(Wq, np.float32).T.astype(bf))
    wkt = np.ascontiguousarray(np.asarray(Wk, np.float32).T.astype(bf))
    wvt = np.ascontiguousarray(np.asarray(Wv, np.float32).T.astype(bf))
    wpt = np.ascontiguousarray(np.asarray(Wp, np.float32).T.astype(bf))
    bpv = np.ascontiguousarray(np.asarray(bp, np.float32).reshape(1, C).astype(bf))

    B, N, _ = x.shape
    in_maps = []
    for core in range(8):
        b, qh = core // 2, core % 2
        xtc = np.ascontiguousarray(x[b].T.astype(bf))
        xqc = np.ascontiguousarray(xtc[:, qh * NQ:(qh + 1) * NQ])
        in_maps.append({
            "xt": xtc, "xq": xqc,
            "wqt": wqt, "wkt": wkt, "wvt": wvt, "wpt": wpt, "bp": bpv,
        })

    res = run_bass_kernel_spmd(
        _get_prog(), in_maps, core_ids=list(range(8)),
        trace=bool(os.environ.get("BASS_TRACE")),
    )
    LAST_RESULT = res

    out = np.empty((B, N, C), np.float32)
    for core in range(8):
        b, qh = core // 2, core % 2
        out[b, qh * NQ:(qh + 1) * NQ, :] = res.results[core]["y"]
    return out


# revision 11
# speedup vs baseline: 1.2970x; 1.0104x over previous
"""Multi-head attention (B=4, N=2048, C=768, H=12) on 8 trn2 NeuronCores.

Sharding: core c handles batch b = c//2 and query rows [ (c%2)*1024, +1024 ).

Per-core engine plan (v2):
  PE     : QKV/proj linears, QK^T (2 heads packed on row groups), PV with the
           two q-chunks packed on column groups (full 128-wide array), ones
           matmuls accumulating softmax denominators on psum rows 0/32/64/96,
           reciprocal-broadcast matmuls.
  ScalarE: exp for even heads (table exp), denominator reciprocal via
           ln -> exp(-x), psum->sbuf drains of the linears.
  VectorE: exp for odd heads via Schraudolph int16 bit-trick (one
           tensor_scalar: i16 = st*A + B, bitcast bf16), attention drains,
           normalize muls.
  DMA    : host-side bf16 inputs; sbuf->sbuf shuffles assemble the normalized
           attention output into projection layout during attention.
  Emission interleaves V (pair 0) and next pair's Q/K (pairs 1-4) into the
  attention loop so the PE never idles and HAM stays at full clock.

PSUM budget (8 banks): st 2x[128,1024] = 4, t 2x[128,512] = 2, den 1, rb 1.
"""

import os
import sys

import numpy as np
import ml_dtypes

sys.path.insert(0, "/opt/trn_rl_repo")

import concourse.bass as bass
from concourse import bacc
import concourse.mybir as mybir
from concourse.tile import TileContext
from concourse.bass_utils import run_bass_kernel_spmd
from concourse.dma_utils import dma_copy

P = 128
C = 768
NK = 2048
NQ = 1024
H = 12
DH = 64
CT = C // P          # 6 c-tiles (contraction tiles for the linears)
KT = NK // P         # 16 key tiles
QCH = 512            # q-chunk (max psum bank free dim for fp32)
SCALE = DH ** -0.5
F32 = mybir.dt.float32
BF16 = mybir.dt.bfloat16
I16 = mybir.dt.int16
EXP = mybir.ActivationFunctionType.Exp
LN = mybir.ActivationFunctionType.Ln

LOG2E = 1.4426950408889634
A16 = 128.0 * LOG2E * SCALE          # fold the 1/8 attention scale
B16 = 127.0 * 128.0 - 0.043677448 * 128.0

LAST_RESULT = None
_PROG = None


def _build_program() -> bass.Bass:
    nc = bacc.Bacc(None, target_bir_lowering=False)

    # host supplies bf16 (halves DMA bytes; matches baseline numerics)
    wqt = nc.dram_tensor("wqt", [C, C], BF16, kind="ExternalInput")
    wkt = nc.dram_tensor("wkt", [C, C], BF16, kind="ExternalInput")
    xq = nc.dram_tensor("xq", [C, NQ], BF16, kind="ExternalInput")
    xt = nc.dram_tensor("xt", [C, NK], BF16, kind="ExternalInput")
    wvt = nc.dram_tensor("wvt", [C, C], BF16, kind="ExternalInput")
    wpt = nc.dram_tensor("wpt", [C, C], BF16, kind="ExternalInput")
    bp = nc.dram_tensor("bp", [1, C], BF16, kind="ExternalInput")
    y = nc.dram_tensor("y", [NQ, C], F32, kind="ExternalOutput")

    with TileContext(nc) as tc:
        with (
            tc.tile_pool(name="persist", bufs=1) as persist,
            tc.tile_pool(name="pt0p", bufs=2) as pt0p,
            tc.tile_pool(name="pt1p", bufs=2) as pt1p,
            tc.tile_pool(name="small", bufs=2) as small,
            tc.tile_pool(name="ysb", bufs=2) as ysb,
            tc.tile_pool(name="ps_st", bufs=2, space="PSUM") as ps_st,
            tc.tile_pool(name="ps_t", bufs=2, space="PSUM") as ps_t,
            tc.tile_pool(name="ps_den", bufs=1, space="PSUM") as ps_den,
            tc.tile_pool(name="ps_rb", bufs=1, space="PSUM") as ps_rb,
        ):
            # ---- load inputs (DMA order = dependency order) ----
            def load(cols, tag):
                return [
                    persist.tile([P, cols], BF16, tag=f"{tag}{i}", name=f"{tag}{i}")
                    for i in range(CT)
                ]

            def dma_tiles(tiles, dram, cols, col0=0):
                for i, t in enumerate(tiles):
                    dma_copy(
                        nc.gpsimd,
                        t[:, col0:col0 + cols],
                        dram[i * P:(i + 1) * P, col0:col0 + cols],
                    )

            wqb = load(C, "wqb")
            wkb = load(C, "wkb")
            xqb = load(NQ, "xqb")
            xtb = load(NK, "xtb")
            wvb = load(C, "wvb")
            wpb = load(C, "wpb")

            dma_tiles(wqb, wqt, C)
            dma_tiles(xqb, xq, NQ)
            dma_tiles(wkb, wkt, C)
            dma_tiles(xtb, xt, NQ, col0=0)      # first half of keys
            dma_tiles(xtb, xt, NQ, col0=NQ)     # second half
            dma_tiles(wvb, wvt, C)
            dma_tiles(wpb, wpt, C)

            bpb = persist.tile([1, C], BF16, tag="bpb")
            dma_copy(nc.gpsimd, bpb[:, :], bp[:, :])

            ones = persist.tile([P, P], BF16, tag="ones")
            nc.gpsimd.memset(ones[:, :], 1.0)

            qtb = [persist.tile([P, NQ], BF16, tag=f"qt{i}", name=f"qt{i}") for i in range(CT)]
            ktb = [persist.tile([P, NK], BF16, tag=f"kt{i}", name=f"kt{i}") for i in range(CT)]
            vtb = [persist.tile([P, C], BF16, tag=f"v{i}", name=f"v{i}") for i in range(KT)]
            otb = [persist.tile([P, NQ], BF16, tag=f"ot{i}", name=f"ot{i}") for i in range(CT)]

            def emit_q(hp, pool):
                # qtb[hp] = Wq rows [hp*128, +128) @ xq  -> [128, 1024]
                for j in range(2):
                    ps = pool.tile([P, QCH], F32, tag="st")
                    for k in range(CT):
                        nc.tensor.matmul(
                            ps[:, :],
                            lhsT=wqb[k][:, hp * P:(hp + 1) * P],
                            rhs=xqb[k][:, j * QCH:(j + 1) * QCH],
                            start=(k == 0), stop=(k == CT - 1),
                        )
                    nc.scalar.copy(qtb[hp][:, j * QCH:(j + 1) * QCH], ps[:, :])

            def emit_k_chunk(hp, j, pool):
                ps = pool.tile([P, QCH], F32, tag="st")
                for k in range(CT):
                    nc.tensor.matmul(
                        ps[:, :],
                        lhsT=wkb[k][:, hp * P:(hp + 1) * P],
                        rhs=xtb[k][:, j * QCH:(j + 1) * QCH],
                        start=(k == 0), stop=(k == CT - 1),
                    )
                nc.scalar.copy(ktb[hp][:, j * QCH:(j + 1) * QCH], ps[:, :])

            def emit_v(i, pool):
                # vtb[i] = x rows [i*128,+128) @ Wv.T -> [128, 768]
                for (c0, csz) in ((0, QCH), (QCH, C - QCH)):
                    ps = pool.tile([P, QCH], F32, tag="st")
                    for k in range(CT):
                        nc.tensor.matmul(
                            ps[:, 0:csz],
                            lhsT=xtb[k][:, i * P:(i + 1) * P],
                            rhs=wvb[k][:, c0:c0 + csz],
                            start=(k == 0), stop=(k == CT - 1),
                        )
                    nc.scalar.copy(vtb[i][:, c0:c0 + csz], ps[:, 0:csz])

            # ---- initial linears: Q/K pairs 0,1 + V tile 0 (DMA-bound phase) ----
            emit_q(0, ps_st)
            for j in range(4):
                emit_k_chunk(0, j, ps_st)
            emit_v(0, ps_st)
            emit_q(1, ps_st)
            for j in range(4):
                emit_k_chunk(1, j, ps_st)

            ypart = [
                persist.tile([P, QCH], F32, tag=f"yp{i}", name=f"yp{i}")
                for i in range(16)
            ]

            def emit_proj_part(qi, ch):
                # first 4 head-pair contributions + bias for q-tile qi, chunk ch
                c0, csz = (0, QCH) if ch == 0 else (QCH, C - QCH)
                ps = ps_rb.tile([P, QCH], F32, tag="st", name="pp")
                for t in range(4):
                    nc.tensor.matmul(
                        ps[:, 0:csz],
                        lhsT=otb[t][:, qi * P:(qi + 1) * P],
                        rhs=wpb[t][:, c0:c0 + csz],
                        start=(t == 0), stop=False,
                    )
                nc.tensor.matmul(
                    ps[:, 0:csz],
                    lhsT=ones[0:1, 0:P],
                    rhs=bpb[0:1, c0:c0 + csz],
                    start=False, stop=True, tile_position=(0, 0),
                )
                nc.scalar.copy(ypart[2 * qi + ch][:, 0:csz], ps[:, 0:csz])

            def make_tail_deferred(hp, dsb, tmp0, tmp1):
                # reciprocal (chunked on DVE), broadcast, normalize, assemble —
                # sprinkled into the NEXT pair's k-tile slots
                state = {}

                def recip_chunk(c):
                    if "recb" not in state:
                        state["recb"] = small.tile(
                            [97, QCH], BF16, tag="recb", name="recb"
                        )
                    with nc.allow_low_precision(
                        reason="1/den as bf16 multiplier, matches baseline"
                    ):
                        nc.vector.reciprocal(
                            state["recb"][:, c * DH:(c + 1) * DH],
                            dsb[:, c * DH:(c + 1) * DH],
                        )

                def finish():
                    recb = state["recb"]
                    for h, tmp in ((0, tmp0), (1, tmp1)):
                        rb = ps_rb.tile([P, QCH], F32, tag="st", name="rb")
                        r0, r1 = (0, 32) if h == 0 else (64, 96)
                        nc.tensor.matmul(
                            rb[0:DH, :], lhsT=ones[r0:r0 + 1, 0:DH],
                            rhs=recb[r0:r0 + 1, :],
                            start=True, stop=True, tile_position=(r0, 0),
                        )
                        nc.tensor.matmul(
                            rb[DH:P, :], lhsT=ones[r1:r1 + 1, 0:DH],
                            rhs=recb[r1:r1 + 1, :],
                            start=True, stop=True, tile_position=(r1, 64),
                        )
                        osb = small.tile([P, QCH], BF16, tag="osb", name="osb")
                        nc.vector.tensor_mul(osb[:, :], tmp[:, :], rb[:, :])
                        nc.sync.dma_start(
                            out=otb[hp][h * DH:(h + 1) * DH, 0:QCH],
                            in_=osb[0:DH, :],
                        )
                        nc.sync.dma_start(
                            out=otb[hp][h * DH:(h + 1) * DH, QCH:NQ],
                            in_=osb[DH:P, :],
                        )

                fills = [lambda c=c: recip_chunk(c) for c in range(8)]
                fills.append(finish)
                return fills

            def make_fillers(hp, deferred):
                pe = []
                if hp == 0:
                    for i in range(1, KT):
                        pe.append(lambda i=i: emit_v(i, ps_rb))
                elif hp < CT - 1:
                    nxt = hp + 1
                    pe.append(lambda: emit_q(nxt, ps_rb))
                    for j in range(4):
                        pe.append(lambda j=j: emit_k_chunk(nxt, j, ps_rb))
                else:
                    for qi in range(NQ // P):
                        for ch in range(2):
                            pe.append(
                                lambda qi=qi, ch=ch: emit_proj_part(qi, ch)
                            )
                # interleave deferred tail (engine-queue work) with PE fillers
                fills = []
                a, b = list(deferred), list(pe)
                while a or b:
                    if a:
                        fills.append(a.pop(0))
                    if b:
                        fills.append(b.pop(0))
                return fills

            deferred = []
            final_tail = None
            for hp in range(CT):
                h0, h1 = 2 * hp, 2 * hp + 1
                fills = make_fillers(hp, deferred)
                nf = len(fills)

                den = ps_den.tile([97, QCH], F32, tag="den")
                t_h0 = ps_t.tile([P, QCH], F32, tag="t")
                t_h1 = ps_t.tile([P, QCH], F32, tag="t")

                def av_den(i, pt0, pt1_i):
                    first, last = (i == 0), (i == KT - 1)
                    v0 = vtb[i][:, h0 * DH:(h0 + 1) * DH]
                    v1 = vtb[i][:, h1 * DH:(h1 + 1) * DH]

                    def pch(h, c):
                        sl = slice(c * QCH, (c + 1) * QCH)
                        if h == 0:
                            return pt0[:, sl]
                        return pt1_i[:, sl].bitcast(BF16)

                    nc.tensor.matmul(
                        t_h0[0:DH, :], lhsT=v0, rhs=pch(0, 0),
                        start=first, stop=last, tile_position=(0, 0),
                    )
                    nc.tensor.matmul(
                        t_h0[DH:P, :], lhsT=v0, rhs=pch(0, 1),
                        start=first, stop=last, tile_position=(0, 64),
                    )
                    nc.tensor.matmul(
                        t_h1[0:DH, :], lhsT=v1, rhs=pch(1, 0),
                        start=first, stop=last, tile_position=(0, 0),
                    )
                    nc.tensor.matmul(
                        t_h1[DH:P, :], lhsT=v1, rhs=pch(1, 1),
                        start=first, stop=last, tile_position=(0, 64),
                    )
                    for pos, (h, c) in zip(
                        (0, 32, 64, 96), ((0, 0), (0, 1), (1, 0), (1, 1))
                    ):
                        nc.tensor.matmul(
                            den[pos:pos + 1, :],
                            lhsT=ones[:, pos:pos + 1],
                            rhs=pch(h, c),
                            start=first, stop=last, tile_position=(0, pos),
                        )

                pending = None   # (i, pt0, pt1_i) awaiting AV + den
                for i in range(KT):
                    for f in range(i * nf // KT, (i + 1) * nf // KT):
                        fills[f]()
                    st1 = ps_st.tile([P, NQ], F32, tag="st")
                    st0 = ps_st.tile([P, NQ], F32, tag="st")
                    for j in range(2):
                        nc.tensor.matmul(
                            st1[:, j * QCH:(j + 1) * QCH],
                            lhsT=ktb[hp][DH:P, i * P:(i + 1) * P],
                            rhs=qtb[hp][DH:P, j * QCH:(j + 1) * QCH],
                            start=True, stop=True,
                            tile_position=(64, 0),
                        )
                        nc.tensor.matmul(
                            st0[:, j * QCH:(j + 1) * QCH],
                            lhsT=ktb[hp][0:DH, i * P:(i + 1) * P],
                            rhs=qtb[hp][0:DH, j * QCH:(j + 1) * QCH],
                            start=True, stop=True,
                            tile_position=(0, 0),
                        )
                    if pending is not None:
                        av_den(*pending)
                    pt1_i = pt1p.tile([P, NQ], I16, tag="pt1")
                    nc.vector.tensor_scalar(
                        pt1_i[:, :], st1[:, :], A16, B16,
                        mybir.AluOpType.mult, mybir.AluOpType.add,
                    )
                    pt0 = pt0p.tile([P, NQ], BF16, tag="pt0")
                    nc.scalar.activation(pt0[:, :], st0[:, :], EXP, scale=SCALE)
                    pending = (i, pt0, pt1_i)
                av_den(*pending)

                # immediate tail: release psum banks quickly (no table loads)
                dsb = small.tile([97, QCH], F32, tag="dsb")
                nc.scalar.copy(dsb[:, :], den[:, :])
                tmp0 = small.tile([P, QCH], F32, tag="tmp0")
                nc.scalar.copy(tmp0[:, :], t_h0[:, :])
                tmp1 = small.tile([P, QCH], F32, tag="tmp1")
                nc.vector.tensor_copy(tmp1[:, :], t_h1[:, :])
                deferred = make_tail_deferred(hp, dsb, tmp0, tmp1)

            # last pair's tail runs right here (nothing left to overlap with)
            for f in deferred:
                f()

            # ---- projection finish: add head-pairs 4,5 to the partials ----
            for qi in range(NQ // P):
                for ch in range(2):
                    c0, csz = (0, QCH) if ch == 0 else (QCH, C - QCH)
                    ps = ps_st.tile([P, QCH], F32, tag="st")
                    nc.tensor.matmul(
                        ps[:, 0:csz],
                        lhsT=otb[4][:, qi * P:(qi + 1) * P],
                        rhs=wpb[4][:, c0:c0 + csz],
                        start=True, stop=False,
                    )
                    nc.tensor.matmul(
                        ps[:, 0:csz],
                        lhsT=otb[5][:, qi * P:(qi + 1) * P],
                        rhs=wpb[5][:, c0:c0 + csz],
                        start=False, stop=True,
                    )
                    yt = ysb.tile([P, QCH], F32, tag="y")
                    nc.vector.tensor_add(
                        yt[:, 0:csz], ypart[2 * qi + ch][:, 0:csz], ps[:, 0:csz]
                    )
                    nc.sync.dma_start(
                        out=y[qi * P:(qi + 1) * P, c0:c0 + csz],
                        in_=yt[:, 0:csz],
                    )

    nc.compile()
    return nc


def _get_prog() -> bass.Bass:
    global _PROG
    if _PROG is None:
        _PROG = _build_program()
    return _PROG


def kernel(x, Wq, Wk, Wv, Wp, bp):
    global LAST_RESULT
    bf = ml_dtypes.bfloat16
    x = np.asarray(x, dtype=np.float32)
    wqt = np.ascontiguousarray(np.asarray(Wq, np.float32).T.astype(bf))
    wkt = np.ascontiguousarray(np.asarray(Wk, np.float32).T.astype(bf))
    wvt = np.ascontiguousarray(np.asarray(Wv, np.float32).T.astype(bf))
    wpt = np.ascontiguousarray(np.asarray(Wp, np.float32).T.astype(bf))
    bpv = np.ascontiguousarray(np.asarray(bp, np.float32).reshape(1, C).astype(bf))

    B, N, _ = x.shape
    in_maps = []
    for core in range(8):
        b, qh = core // 2, core % 2
        xtc = np.ascontiguousarray(x[b].T.astype(bf))
        xqc = np.ascontiguousarray(xtc[:, qh * NQ:(qh + 1) * NQ])
        in_maps.append({
            "xt": xtc, "xq": xqc,
            "wqt": wqt, "wkt": wkt, "wvt": wvt, "wpt": wpt, "bp": bpv,
        })

    res = run_bass_kernel_spmd(
        _get_prog(), in_maps, core_ids=list(range(8)),
        trace=bool(os.environ.get("BASS_TRACE")),
    )
    LAST_RESULT = res

    out = np.empty((B, N, C), np.float32)
    for core in range(8):
        b, qh = core // 2, core % 2
        out[b, qh * NQ:(qh + 1) * NQ, :] = res.results[core]["y"]
    return out
